# revision 71
# baseline (speedup 1.0000x reference)
"""Causal self-attention (GQA 16q/4kv, rms_norm + rope + q_gain) on 8 trn2 cores.

Sharding: tensor-parallel over heads. Core c owns q-heads {2c, 2c+1} and kv-head
c//2. Each core computes its heads' attention output y_h^T [128, S] and a
partial projection partial_c^T [1024, S]; the host sums the 8 partials and
transposes back to [1, S, 1024].

Schedule (per core): the per-kb chain St -> exp -> AV is software-pipelined
(St(kb+1) issued before AV(kb)) so PE fills exp latency; qkv projection, rope,
and deferred proj of the previous chunk are broken into small filler tasks
popped between kb slots. rsqrt for rms_norm is exp(-0.5*ln(ms)) so the ACT
table (exp/ln/square in one set) is never reloaded; q_gain folds into the
exp bias as ln(gain). Causal diagonal = resident bf16 triangle-mask multiply.

Layout (per core, d-major = feature dim on partitions):
  Qt  [128, S] f32r  rows = 2 heads x 64 dims  (rms-normed, roped, gained)
  Kt2 [128, S] f32r  kv head duplicated to both partition halves
  V   [128, NKB, 80] bf16  seq-major tiles [128, 65] (col 64 = ones -> denom)
  St block = Kt_blk.T @ Qt_chunk -> [128 k, 2x512 q] psum, exp on ACT -> bf16
  AV: y^T[65, 512] += Vones_blk.T @ St_exp  (row 64 = softmax denom)
"""

import sys

for _p in ("/opt/trn_rl_repo",):
    if _p not in sys.path:
        sys.path.insert(0, _p)

import numpy as np
from contextlib import ExitStack

import concourse.bass as bass
import concourse.tile as tile
from concourse import mybir
from concourse.bass_utils import run_bass_kernel_spmd

F32 = mybir.dt.float32
F32R = mybir.dt.float32r
BF16 = mybir.dt.bfloat16
AF = mybir.ActivationFunctionType

DIM = 1024
NUM_HEADS = 16
NUM_KV_HEADS = 4
HD = 64
ROPE_BASE = 10000.0
RMS_EPS = 1.1920929e-07
N_CORES = 8

SCALE = 1.0 / 8.0  # 1/sqrt(64)


def build_nc(S: int, split: bool = True, iters: int = 1) -> bass.Bass:
    """Build the per-core SPMD Bass program for sequence length S (mult of 512)."""
    assert S % 512 == 0
    NCH = S // 512          # 512-wide seq chunks
    NKB = S // 128          # 128-wide k blocks

    nc = bass.Bass("TRN2", debug=False)

    xt_d = nc.declare_dram_parameter("xt", [DIM, S], F32R, isOutput=False)
    wqkv_d = nc.declare_dram_parameter("wqkv_t", [DIM, 256], F32R, isOutput=False)
    wproj_d = nc.declare_dram_parameter("wproj_t", [128, DIM], F32R, isOutput=False)
    cos_d = nc.declare_dram_parameter("cos1", [32, S], F32, isOutput=False)
    sin_d = nc.declare_dram_parameter("sin2s", [64, S], F32, isOutput=False)
    lgain_d = nc.declare_dram_parameter("lgain", [128, 1], F32, isOutput=False)
    out_d = nc.declare_dram_parameter("out", [DIM, S], BF16, isOutput=True)

    with tile.TileContext(nc) as tc, ExitStack() as ctx:
        res = ctx.enter_context(tc.tile_pool(name="res", bufs=1))
        xtp = ctx.enter_context(tc.tile_pool(name="xtp", bufs=2))
        ropep = ctx.enter_context(tc.tile_pool(name="ropep", bufs=1))
        qsbp = ctx.enter_context(tc.tile_pool(name="qsbp", bufs=2))
        rowp = ctx.enter_context(tc.tile_pool(name="rowp", bufs=2))
        vtp = ctx.enter_context(tc.tile_pool(name="vtp", bufs=2))
        sep = ctx.enter_context(tc.tile_pool(name="sep", bufs=6))
        outp = ctx.enter_context(tc.tile_pool(name="outp", bufs=4))
        pst = ctx.enter_context(tc.tile_pool(name="pst", bufs=2, space="PSUM"))
        pyp = ctx.enter_context(tc.tile_pool(name="pyp", bufs=1, space="PSUM"))
        # ring-2 on [128,512] tiles (2 banks): the former joint [128,1024]
        # ring-1 serialized every msp/vtr/po/rps consumer into one chain
        pms = ctx.enter_context(tc.tile_pool(name="pms", bufs=2, space="PSUM"))

        # ---- resident tiles ----
        qt_sb = res.tile([128, S], F32R, tag="qt", name="qt_sb")
        kt2_sb = res.tile([128, S], F32R, tag="kt2", name="kt2_sb")
        yt_sb = res.tile([128, S], F32R, tag="yt", name="yt_sb")
        v_sb = res.tile([128, NKB, 80], BF16, tag="v", name="v_sb")
        wqkv_sb = res.tile([128, 8, 256], F32R, tag="wqkv", name="wqkv_sb")
        wproj_sb = res.tile([128, DIM], F32R, tag="wproj", name="wproj_sb")
        cos_sb = res.tile([128, S], F32, tag="cos", name="cos_sb")
        sin_sb = res.tile([128, S], F32, tag="sin", name="sin_sb")
        lgain_sb = res.tile([128, 1], F32, tag="lgain", name="lgain_sb")
        onesbd = res.tile([128, 128], BF16, tag="onesbd", name="onesbd")
        ones_k = res.tile([64, 128], BF16, tag="onesk", name="ones_k")
        id64 = res.tile([64, 64], BF16, tag="id64", name="id64")
        trimask = res.tile([128, 128], BF16, tag="trim", name="trimask")
        ones_r = res.tile([1, 64], F32R, tag="onesr", name="ones_r")
        const_sb = res.tile([128, 2], F32, tag="const", name="const_sb")
        rrn = res.tile([64, 1024], F32, tag="rrn", name="rrn")

        # ---- one-time setup ----
        # wqkv on the ACT hwdge queue (in parallel with the SP queue's xtile-0
        # prefetch, both needed before the first qkv matmul); cos/sin after.
        for dt in range(8):
            nc.scalar.dma_start(out=wqkv_sb[:, dt, :], in_=wqkv_d[dt * 128:(dt + 1) * 128, :])
        # cos: host sends [32,S] -> 4x tiled to [128,S] via 0-stride src AP;
        # sin: host sends [64,S]=[sin;-sin] -> 2x tiled (quadrant signs).
        cos_src1 = bass.AP(cos_d[:].tensor, 0, [[0, 4], [S, 32], [1, 512]])
        cos_src2 = bass.AP(cos_d[:].tensor, 512, [[0, 4], [S, 32], [1, S - 512]])
        sin_src1 = bass.AP(sin_d[:].tensor, 0, [[0, 2], [S, 64], [1, 512]])
        sin_src2 = bass.AP(sin_d[:].tensor, 512, [[0, 2], [S, 64], [1, S - 512]])
        nc.scalar.dma_start(out=cos_sb[:, 0:512], in_=cos_src1)
        nc.scalar.dma_start(out=sin_sb[:, 0:512], in_=sin_src1)
        nc.scalar.dma_start(out=lgain_sb[:], in_=lgain_d[:])
        nc.scalar.dma_start(out=cos_sb[:, 512:S], in_=cos_src2)
        nc.scalar.dma_start(out=sin_sb[:, 512:S], in_=sin_src2)
        nc.scalar.dma_start(out=wproj_sb[:], in_=wproj_d[:])
        nc.vector.memset(onesbd[:], 0.0)
        nc.vector.memset(onesbd[0:64, 0:64], 1.0)
        nc.vector.memset(onesbd[64:128, 64:128], 1.0)
        nc.vector.memset(ones_k[:], 1.0)
        # 64x64 identity: ones, then keep col>=p, then keep p>=col
        nc.vector.memset(id64[:], 1.0)
        nc.gpsimd.affine_select(
            out=id64[:], in_=id64[:], compare_op=mybir.AluOpType.is_ge,
            fill=0.0, base=0, pattern=[[1, 64]], channel_multiplier=-1)
        nc.gpsimd.affine_select(
            out=id64[:], in_=id64[:], compare_op=mybir.AluOpType.is_ge,
            fill=0.0, base=0, pattern=[[-1, 64]], channel_multiplier=1)
        nc.vector.memset(trimask[:], 1.0)
        # keep col >= partition (causal lower triangle in k-major layout)
        nc.gpsimd.affine_select(
            out=trimask[:], in_=trimask[:], compare_op=mybir.AluOpType.is_ge,
            fill=0.0, base=0, pattern=[[1, 128]], channel_multiplier=-1)
        # memset can't target f32r; copy from the bf16 ones block instead
        nc.vector.tensor_copy(ones_r[:], onesbd[0:1, 0:64])
        nc.vector.memset(const_sb[:, 0:1], 0.0)
        nc.vector.memset(const_sb[:, 1:2], RMS_EPS)
        zb128 = const_sb[:, 0:1]          # zero bias, 128 partitions
        zb64 = const_sb[0:64, 0:1]
        eps128 = const_sb[:, 1:2]
        eps64 = const_sb[0:64, 1:2]
        # ones column of every V tile (col 64 of each 80-wide block)
        nc.vector.memset(v_sb[:, :, 64:65], 1.0)

        def _chunks():
            xtiles = {}

            def prefetch_x(c):
                if c >= NCH or c in xtiles:
                    return
                xtile = xtp.tile([128, 8, 512], F32R, tag="xt", name="xtile")
                # two half DMAs: subtile deps let the first qkv matmuls start
                # after the first half lands (matters in the prologue)
                for h in range(2):
                    xt_src = bass.AP(xt_d[:].tensor, c * 512 + h * 4 * 128 * S,
                                     [[S, 128], [128 * S, 4], [1, 512]])
                    nc.sync.dma_start(out=xtile[:, 4 * h:4 * h + 4, :], in_=xt_src)
                xtiles[c] = xtile

            def make_qkv_tasks(c):
                """Filler tasks computing chunk c's q/k/v + rope; run during
                chunk c-1. Small pieces so no engine sees a long lump."""
                s0 = c * 512
                sl = slice(s0, s0 + 512)
                st = {}

                def t_prefetch():
                    prefetch_x(c + 1)

                def t_q_mm():
                    qkv_ps = pst.tile([128, 1024], F32, tag="st", name="qkv_ps")
                    st["ps"] = qkv_ps
                    for dt in range(8):
                        nc.tensor.matmul(qkv_ps[:, 0:512],
                                         lhsT=wqkv_sb[:, dt, 0:128],
                                         rhs=xtiles[c][:, dt, :],
                                         start=(dt == 0), stop=(dt == 7))

                def t_kv_mm():
                    qkv_ps = st["ps"]
                    for dt in range(8):
                        nc.tensor.matmul(qkv_ps[:, 512:1024],
                                         lhsT=wqkv_sb[:, dt, 128:256],
                                         rhs=xtiles[c][:, dt, :],
                                         start=(dt == 0), stop=(dt == 7))
                    xtiles.pop(c)

                def t_unload():
                    # free the psum ring slot fast: copy q/kv to SBUF.
                    # DVE, not Pool: GPSIMD instructions cannot access PSUM.
                    qkv_ps = st["ps"]
                    q_sb = qsbp.tile([128, 512], F32, tag="q", name="q_sb")
                    kv_sb = qsbp.tile([128, 512], F32, tag="kv", name="kv_sb")
                    nc.vector.tensor_copy(q_sb[:], qkv_ps[:, 0:512])
                    nc.vector.tensor_copy(kv_sb[:], qkv_ps[:, 512:1024])
                    st["q"] = q_sb
                    st["kv"] = kv_sb

                def t_sq():
                    q_sb, kv_sb = st["q"], st["kv"]
                    sq_q = ropep.tile([128, 512], BF16, tag="sqq", name="sq_q")
                    nc.vector.tensor_mul(sq_q[:], q_sb[:], q_sb[:])
                    sq_k = ropep.tile([64, 512], BF16, tag="sqk", name="sq_k")
                    # DVE, not Pool: msk's matmul was stalling ~1.3us/chunk
                    # behind Pool's rope backlog waiting for this square
                    nc.vector.tensor_mul(sq_k[:], kv_sb[0:64, :], kv_sb[0:64, :])
                    st["sq"] = (sq_q, sq_k)

                def t_ms():
                    # mean-square sums (separate filler from t_sq: the mms sit
                    # at the end of a DVE chain and would stall PE otherwise)
                    sq_q, sq_k = st["sq"]
                    msp = pms.tile([128, 512], F32, tag="b1", name="msp")
                    nc.tensor.matmul(msp[:], lhsT=onesbd[:], rhs=sq_q[:],
                                     start=True, stop=True)
                    msk = pms.tile([64, 512], F32, tag="b1", name="msk")
                    nc.tensor.matmul(msk[:], lhsT=onesbd[0:64, 0:64], rhs=sq_k[:],
                                     start=True, stop=True)
                    st["ms"] = (msp, msk)

                def t_fb():
                    # rsqrt(ms+eps) = exp(-0.5*ln(ms+eps)); gain folds into the
                    # exp bias as ln(gain). Exp/Ln/Square share one ACT table.
                    msp, msk = st["ms"]
                    lq = ropep.tile([128, 512], F32, tag="lq", name="lq")
                    nc.scalar.activation(lq[:], msp[:], AF.Ln, bias=eps128,
                                         scale=1.0 / HD)
                    lk = ropep.tile([64, 512], F32, tag="lk", name="lk")
                    nc.scalar.activation(lk[:], msk[:], AF.Ln, bias=eps64,
                                         scale=1.0 / HD)
                    fbq = ropep.tile([128, 512], F32, tag="fbq", name="fbq")
                    nc.scalar.activation(fbq[:], lq[:], AF.Exp,
                                         bias=lgain_sb[:, 0:1], scale=-0.5)
                    fbk = ropep.tile([64, 512], F32, tag="fbk", name="fbk")
                    nc.scalar.activation(fbk[:], lk[:], AF.Exp, bias=zb64,
                                         scale=-0.5)
                    st["fbq"] = fbq
                    st["fbk"] = fbk

                def t_rope_qa():
                    # head A (rows 0:64) on DVE: swap-copy rotate-half, then
                    # aligned muls (binary ops need equal input base partition)
                    q_sb, fbq = st["q"], st["fbq"]
                    # ring-2: in the zipped prologue, chunk 1's DVE rope
                    # otherwise waits on chunk 0's Pool half via this ring
                    qsw = ropep.tile([128, 512], F32, tag="qsw", name="qsw", bufs=2)
                    tq = ropep.tile([128, 512], F32, tag="tq", name="tq", bufs=2)
                    st["qsw"], st["tq"] = qsw, tq
                    nc.vector.tensor_copy(qsw[0:32, :], q_sb[32:64, :])
                    nc.vector.tensor_copy(qsw[32:64, :], q_sb[0:32, :])
                    nc.vector.tensor_mul(tq[0:64, :], q_sb[0:64, :], cos_sb[0:64, sl])
                    nc.vector.tensor_mul(qsw[0:64, :], qsw[0:64, :], sin_sb[0:64, sl])
                    nc.vector.tensor_add(tq[0:64, :], tq[0:64, :], qsw[0:64, :])
                    nc.vector.tensor_mul(qt_sb[0:64, sl], tq[0:64, :], fbq[0:64, :])

                def t_rope_qb():
                    # head B (rows 64:128) on Pool
                    q_sb, fbq = st["q"], st["fbq"]
                    qsw, tq = st["qsw"], st["tq"]
                    nc.gpsimd.tensor_copy(qsw[64:96, :], q_sb[96:128, :])
                    nc.gpsimd.tensor_copy(qsw[96:128, :], q_sb[64:96, :])
                    nc.gpsimd.tensor_mul(tq[64:128, :], q_sb[64:128, :], cos_sb[64:128, sl])
                    nc.gpsimd.tensor_mul(qsw[64:128, :], qsw[64:128, :], sin_sb[64:128, sl])
                    nc.gpsimd.tensor_add(tq[64:128, :], tq[64:128, :], qsw[64:128, :])
                    nc.gpsimd.tensor_mul(qt_sb[64:128, sl], tq[64:128, :], fbq[64:128, :])

                def t_rope_k():
                    # k-rope on DVE (DVE ops ~2x faster than Pool)
                    kv_sb, fbk = st["kv"], st["fbk"]
                    ksw = ropep.tile([64, 512], F32, tag="ksw", name="ksw", bufs=2)
                    tk = ropep.tile([64, 512], F32, tag="tk", name="tk", bufs=2)
                    nc.vector.tensor_copy(ksw[0:32, :], kv_sb[32:64, :])
                    nc.vector.tensor_copy(ksw[32:64, :], kv_sb[0:32, :])
                    nc.vector.tensor_mul(tk[:], kv_sb[0:64, :], cos_sb[0:64, sl])
                    nc.vector.tensor_mul(ksw[:], ksw[:], sin_sb[0:64, sl])
                    nc.vector.tensor_add(tk[:], tk[:], ksw[:])
                    nc.vector.tensor_mul(kt2_sb[0:64, sl], tk[:], fbk[:])

                def t_ktdup_v():
                    # SBUF-to-SBUF: fine on Pool (lead-2 gives ample slack)
                    nc.gpsimd.tensor_copy(kt2_sb[64:96, sl], kt2_sb[0:32, sl])
                    nc.gpsimd.tensor_copy(kt2_sb[96:128, sl], kt2_sb[32:64, sl])
                    kv_sb = st["kv"]
                    vtt = vtp.tile([64, 512], BF16, tag="vt", name="vtt")
                    nc.gpsimd.tensor_copy(vtt[:], kv_sb[64:128, :])
                    st["vt"] = vtt

                def mk_vtr(j):
                    # V transpose on PE: out[pos,d] = sum_k vtt[k,pos]*I[k,d].
                    # (DMA transposes raced across hw queues; PE is tracked.)
                    # One per filler: consecutive vtr mms would block PE on
                    # the ring-1 DVE copy between them.
                    def task():
                        vtt = st["vt"]
                        kb = 4 * c + j
                        vtr = pms.tile([128, 64], F32, tag="b1", name="vtr")
                        nc.tensor.matmul(vtr[:], lhsT=vtt[:, j * 128:(j + 1) * 128],
                                         rhs=id64[:], start=True, stop=True)
                        nc.vector.tensor_copy(v_sb[:, kb, 0:64], vtr[:])
                    return task

                return [t_prefetch, t_q_mm, t_kv_mm,
                        t_unload, t_sq, t_ms, t_fb, t_rope_qa, t_rope_qb,
                        t_rope_k, t_ktdup_v] + [mk_vtr(j) for j in range(4)]

            def st_part(c, kb):
                s0 = c * 512
                j = kb - 4 * c
                qlo = 128 * j if j > 0 else 0
                stp = pst.tile([128, 1024], F32, tag="st", name="stp")
                nc.tensor.matmul(stp[:, qlo:512],
                                 lhsT=kt2_sb[0:64, kb * 128:(kb + 1) * 128],
                                 rhs=qt_sb[0:64, s0 + qlo:s0 + 512],
                                 start=True, stop=True, tile_position=(0, 0))
                nc.tensor.matmul(stp[:, 512 + qlo:1024],
                                 lhsT=kt2_sb[64:128, kb * 128:(kb + 1) * 128],
                                 rhs=qt_sb[64:128, s0 + qlo:s0 + 512],
                                 start=True, stop=True, tile_position=(64, 0))
                se = sep.tile([128, 1024], BF16, tag="se", name="se")
                if qlo == 0:
                    nc.scalar.activation(se[:], stp[:], AF.Exp, bias=zb128,
                                         scale=SCALE)
                else:
                    # one dual-block activation for both heads' partial cols
                    se2 = bass.AP(se.tensor, se.offset + qlo,
                                  [se.ap[0], [512, 2], [1, 512 - qlo]])
                    st2 = bass.AP(stp.tensor, stp.offset + qlo,
                                  [stp.ap[0], [512, 2], [1, 512 - qlo]])
                    nc.scalar.activation(se2, st2, AF.Exp, bias=zb128,
                                         scale=SCALE)
                if j >= 0:
                    # causal mask: only the 128-wide diagonal band needs it
                    se3 = bass.AP(se.tensor, se.offset + qlo,
                                  [se.ap[0], [512, 2], [1, 128]])
                    nc.gpsimd.affine_select(
                        out=se3, in_=se3, compare_op=mybir.AluOpType.is_ge,
                        fill=0.0, base=0, pattern=[[0, 2], [1, 128]],
                        channel_multiplier=-1)
                return (kb, qlo, se)

            def av_part(pend, y0, y1, nkb):
                kb, qlo, se = pend
                first, last = (kb == 0), (kb == nkb - 1)
                nc.tensor.matmul(y0[:, qlo:512], lhsT=v_sb[:, kb, 0:65],
                                 rhs=se[:, qlo:512], start=first, stop=last)
                nc.tensor.matmul(y1[:, qlo:512], lhsT=v_sb[:, kb, 0:65],
                                 rhs=se[:, 512 + qlo:1024], start=first, stop=last)

            def finish_chunk(qc, y0, y1, last=False):
                """Deferred tasks normalizing chunk qc's attention output and
                projecting it. All run as fillers inside chunk qc+1's kb loop
                so PE never pauses at the chunk boundary. Proj partials go
                PSUM -> DRAM directly (f32; host sums partials anyway)."""
                s0 = qc * 512
                sl = slice(s0, s0 + 512)
                fin = {}

                def t_denom():
                    srow = rowp.tile([1, 1024], F32R, tag="row", name="srow")
                    nc.vector.tensor_copy(srow[0:1, 0:512], y0[64:65, :])
                    nc.vector.tensor_copy(srow[0:1, 512:1024], y1[64:65, :])
                    rps0 = pms.tile([64, 512], F32, tag="b1", name="rps0")
                    nc.tensor.matmul(rps0[:], lhsT=ones_r[:],
                                     rhs=srow[0:1, 0:512], start=True, stop=True)
                    rps1 = pms.tile([64, 512], F32, tag="b1", name="rps1")
                    nc.tensor.matmul(rps1[:], lhsT=ones_r[:],
                                     rhs=srow[0:1, 512:1024], start=True, stop=True)
                    fin["rps"] = (rps0, rps1)

                def t_norm():
                    rps0, rps1 = fin["rps"]
                    nc.vector.reciprocal(rrn[:, 0:512], rps0[:])
                    nc.vector.reciprocal(rrn[:, 512:1024], rps1[:])
                    nc.vector.tensor_mul(yt_sb[0:64, sl], y0[0:64, :], rrn[0:64, 0:512])
                    nc.vector.tensor_mul(yt_sb[64:96, sl], y1[0:32, :], rrn[0:32, 512:1024])
                    nc.vector.tensor_mul(yt_sb[96:128, sl], y1[32:64, :], rrn[32:64, 512:1024])

                def mk(ot):
                    def task():
                        if last:
                            # tail: pms is ring-1 (mm->copy serializes); reuse
                            # the now-dead y0/y1 banks for 2-deep pipelining
                            po = pyp.tile([128, 512], F32,
                                          tag=("y0" if ot % 2 == 0 else "y1"),
                                          name="po")
                        else:
                            po = pms.tile([128, 512], F32, tag="b1", name="po")
                        nc.tensor.matmul(po[:], lhsT=wproj_sb[:, ot * 128:(ot + 1) * 128],
                                         rhs=yt_sb[:, sl], start=True, stop=True)
                        # stage + DMA per ot (DVE copy: GPSIMD can't read
                        # PSUM); immediate DMA keeps the kernel tail short
                        otb = outp.tile([128, 512], BF16, tag="ot", name="otb")
                        nc.vector.tensor_copy(otb[:], po[:])
                        nc.sync.dma_start(
                            out=out_d[ot * 128:(ot + 1) * 128, s0:s0 + 512],
                            in_=otb[:])
                    return task

                return [t_denom, t_norm] + [mk(ot) for ot in range(8)]

            # ---------------- main pipeline ----------------
            # lead-2 qkv pipeline: prologue computes chunks 0 and 1; chunk c's
            # fillers compute chunk c+2 (a full chunk of slack before use)
            prefetch_x(0)
            prefetch_x(1)
            # zip the two prologue chains: each stage of chunk 1 issues right
            # after the same stage of chunk 0, pipelining across engines
            t0, t1 = make_qkv_tasks(0), make_qkv_tasks(1)
            for a, b in zip(t0, t1):
                a()
                b()
            proj_tasks = []
            pend = None

            for c in range(NCH):
                nkb = 4 * (c + 1)
                # fillers: finish/proj of chunk c-1 first two (denorm), then
                # qkv of chunk c+1 (bulk PE), then proj mms of c-1
                # Weave fillers: denorm of chunk c-1 interleaved with chunk
                # c+1's qkv matmul lumps in the first two slots (the serial
                # denom->normalize chain, which gates this chunk's first AV
                # via the y ring, hides behind ~3.4us of independent PE work);
                # then the serial qkv chain one task per slot; proj mms after.
                qk = make_qkv_tasks(c + 2) if c + 2 < NCH else []
                if qk and proj_tasks:
                    fl = ([proj_tasks[0], qk[0], qk[1], proj_tasks[1], qk[2]]
                          + qk[3:] + proj_tasks[2:])
                else:
                    fl = qk + proj_tasks
                pops = [0] * nkb
                pops[0] = min(3, len(fl))
                if nkb > 1:
                    pops[1] = min(2, len(fl) - pops[0])
                rest = len(fl) - sum(pops)
                if rest > 0 and nkb > 2:
                    # spread evenly over slots 2..nkb-1 (lead-2 gives the qkv
                    # chain a full chunk of slack, so no front-load needed;
                    # bare slots stall PE ~300ns each on the exp wait)
                    span = nkb - 2
                    for i in range(rest):
                        pops[2 + (i * span) // rest] += 1
                elif rest > 0:
                    pops[0] += rest
                y0 = pyp.tile([65, 512], F32, tag="y0", name="y0")
                y1 = pyp.tile([65, 512], F32, tag="y1", name="y1")
                fi = 0
                for kb in range(nkb):
                    info = st_part(c, kb)
                    # slot 0: the carried-over AV of the previous chunk MUST
                    # issue before t_denom's y read, or the tracker orders
                    # the AV after the read (denominator misses the last kb)
                    if kb == 0 and pend is not None:
                        av_part(*pend)
                        pend = None
                    # fillers between St(kb) and the pending AV: PE stays fed
                    # while the pending exp finishes on ACT (p-state stays up)
                    for _ in range(pops[kb]):
                        fl[fi]()
                        fi += 1
                    if pend is not None:
                        av_part(*pend)
                    # pipeline carries across the chunk boundary: the last
                    # AV of chunk c issues in chunk c+1's first slot, hiding
                    # the denorm chain behind St(c+1, 0) + qkv fillers
                    pend = (info, y0, y1, nkb)
                while fi < len(fl):
                    fl[fi]()
                    fi += 1
                proj_tasks = finish_chunk(c, y0, y1, last=(c == NCH - 1))

            av_part(*pend)
            for t in proj_tasks:
                t()

        if iters > 1:
            with tc.For_i(0, iters, 1) as _i:
                _chunks()
        else:
            _chunks()

    if split:
        split_multi_waits(nc)
    return nc


def split_multi_waits(nc, max_waits=1):
    """walrus's per-instruction sync encoding only fits one sem wait on some
    instruction types (e.g. the matmul LDWEIGHTS struct). Hoist extra waits
    onto same-engine NoOps inserted just before the instruction."""
    nid = [0]
    for fn in nc.m.functions:
        for blk in fn.blocks:
            out = []
            for inst in blk.instructions:
                si = inst.sync_info
                if si is not None and len(si.on_wait) > max_waits:
                    waits = list(si.on_wait)
                    for w in waits[:-max_waits]:
                        nop = mybir.InstNoOp(name=f"waitsplit-{nid[0]}", ins=[], outs=[])
                        nid[0] += 1
                        nop.engine = inst.engine
                        nop.sync_info = mybir.SyncInfo(on_wait=[w], on_update=[])
                        out.append(nop)
                    inst.sync_info = mybir.SyncInfo(on_wait=waits[-max_waits:],
                                                    on_update=list(si.on_update))
                out.append(inst)
            blk.instructions = out


def make_host_inputs(x, Wq, Wk, Wv, Wproj, q_gain, S):
    """Slice/transpose full inputs into per-core in_maps (host-side prep)."""
    xt = np.ascontiguousarray(x.reshape(S, DIM).T).astype(np.float32, copy=False)

    inv_freq = 1.0 / (ROPE_BASE ** (np.arange(0, HD, 2, dtype=np.float32) / HD))
    t = np.arange(S, dtype=np.float32)
    freqs = np.outer(t, inv_freq).astype(np.float32)        # [S, 32]
    cos1 = np.ascontiguousarray(np.cos(freqs).T.astype(np.float32))   # [32, S]
    sin_t = np.sin(freqs).T.astype(np.float32)
    sin2s = np.ascontiguousarray(
        np.concatenate([sin_t, -sin_t], axis=0))            # [64, S]

    in_maps = []
    for c in range(N_CORES):
        kv = c // 2
        wq_c = Wq[128 * c:128 * (c + 1), :]                 # [128, 1024]
        wk_c = Wk[64 * kv:64 * (kv + 1), :]                 # [64, 1024]
        wv_c = Wv[64 * kv:64 * (kv + 1), :]
        wqkv_t = np.ascontiguousarray(
            np.concatenate([wq_c, wk_c, wv_c], axis=0).T).astype(np.float32, copy=False)
        wproj_t = np.ascontiguousarray(
            Wproj[:, 128 * c:128 * (c + 1)].T).astype(np.float32, copy=False)
        gain = np.repeat(q_gain[2 * c:2 * c + 2], 64).reshape(128, 1)
        assert np.all(gain > 0), "q_gain must be positive for ln-gain folding"
        lgain = np.ascontiguousarray(np.log(gain)).astype(np.float32, copy=False)
        in_maps.append({
            "xt": xt,
            "wqkv_t": wqkv_t,
            "wproj_t": wproj_t,
            "cos1": cos1,
            "sin2s": sin2s,
            "lgain": lgain,
        })
    return in_maps


_NC_CACHE = {}


def get_nc(S, iters=1):
    key = (S, iters)
    if key not in _NC_CACHE:
        _NC_CACHE[key] = build_nc(S, iters=iters)
    return _NC_CACHE[key]


def kernel(x, Wq, Wk, Wv, Wproj, q_gain, trace=False):
    x = np.asarray(x, dtype=np.float32)
    B, S, D = x.shape
    assert B == 1 and D == DIM
    in_maps = make_host_inputs(
        x, np.asarray(Wq, np.float32), np.asarray(Wk, np.float32),
        np.asarray(Wv, np.float32), np.asarray(Wproj, np.float32),
        np.asarray(q_gain, np.float32), S)

    nc = get_nc(S)
    r = run_bass_kernel_spmd(nc, in_maps, core_ids=list(range(N_CORES)), trace=trace)
    total = np.zeros((DIM, S), dtype=np.float32)
    for c in range(N_CORES):
        total += np.asarray(r.results[c]["out"]).astype(np.float32)
    out = np.ascontiguousarray(total.T).reshape(1, S, DIM)
    if trace:
        kernel._last_results = r
    return out
